# revision 1
# baseline (speedup 1.0000x reference)
"""Trainium2 Bass kernel for nn_LongTermMemory (retrieval_knn).

reference: cos-sim KNN: best[b] = argmax_m cos(context[b], memory[m]);
return memory[best][None] -> [1, B, D].

Strategy (8 NeuronCores): shard memory [65536, 512] on M -> 8192 rows/core.
Per core (all on device):
  - normalize memory rows (ACT square+accum -> sqrt -> recip), convert to
    bf16, DMA-xbar transpose to get d-on-partitions layout,
  - bf16 matmul sim[b_chunk 128, m 512-groups] against transposed normalized
    context (fp32->bf16 screening),
  - vector-engine max (top-8) + max_index per 4096-segment -> candidate
    indices per (b, segment).
Host: exact fp64 re-rank of the ~128 candidates per b (screening in bf16 is
only used to select candidates; final argmax decided at fp64 precision),
then gather rows. This makes the argmax numerically robust.
"""

import numpy as np
import ml_dtypes

import concourse.bacc as bacc
import concourse.tile as tile
from concourse import mybir
from concourse.bass_utils import run_bass_kernel_spmd

B, D, M_TOT = 512, 512, 65536
C = 8                    # cores
M = M_TOT // C           # 8192 rows per core
P = 128
TB = B // P              # 4 b-chunks
TD = D // P              # 4 d-chunks
TM = M // P              # 64 m-tiles
GM = 4                   # m-tiles per matmul group (N=512 moving)
NG = TM // GM            # 16 groups
Q = 2                    # max segments per b-chunk
SEG = M // Q             # 4096
F32 = mybir.dt.float32
BF16 = mybir.dt.bfloat16
U32 = mybir.dt.uint32

_NC_CACHE = {}


def build_nc(skip=()):
    key = ("nc",) + tuple(sorted(skip))
    if key in _NC_CACHE:
        return _NC_CACHE[key]
    from contextlib import ExitStack

    nc = bacc.Bacc("TRN2", target_bir_lowering=False, debug=False)
    ctx_dram = nc.dram_tensor("ctx", [B, D], F32, kind="ExternalInput")
    mem_dram = nc.dram_tensor("mem", [M, D], F32, kind="ExternalInput")
    eye_dram = nc.dram_tensor("eye", [P, P], BF16, kind="ExternalInput")
    cv_dram = nc.dram_tensor("cand_v", [TB, Q, P, 8], BF16, kind="ExternalOutput")
    ci_dram = nc.dram_tensor("cand_i", [TB, Q, P, 8], U32, kind="ExternalOutput")

    with tile.TileContext(nc) as tc, ExitStack() as ex:
        big = ex.enter_context(tc.tile_pool(name="big", bufs=1))
        mp = ex.enter_context(tc.tile_pool(name="mp", bufs=6))
        sq = ex.enter_context(tc.tile_pool(name="sq", bufs=2))
        nb = ex.enter_context(tc.tile_pool(name="nb", bufs=4))
        sm = ex.enter_context(tc.tile_pool(name="sm", bufs=4))
        ps = ex.enter_context(tc.tile_pool(name="ps", bufs=4, space="PSUM"))
        xs = ex.enter_context(tc.tile_pool(name="xs", bufs=3, space="PSUM"))

        # persistent buffers
        ctxT = big.tile([P, TB, TD, P], BF16)        # [d_low, beta, j, b_low]
        memT = big.tile([P, TM, TD, P], BF16)        # [d_low, t, j, m_low]
        simb = big.tile([P, TB, M], BF16)            # [b_low, beta, m]
        ssq = big.tile([P, TM], F32)
        srt = big.tile([P, TM], F32)
        rin = big.tile([P, TM], F32)

        eye = big.tile([P, P], BF16)
        nc.sync.dma_start(eye[:], eye_dram[:])

        # ---- context prep: normalize + bf16 + transpose ----
        for b in range(TB):
            cf = mp.tile([P, D], F32, tag="cf")
            nc.sync.dma_start(cf[:], ctx_dram[b * P:(b + 1) * P, :])
            csq = sq.tile([P, 1], F32, tag="csq")
            cdump = sq.tile([P, D], BF16, tag="cdump")
            nc.scalar.activation(cdump[:], cf[:],
                                 mybir.ActivationFunctionType.Square,
                                 accum_out=csq[:])
            csr = sq.tile([P, 1], F32, tag="csr")
            nc.scalar.sqrt(csr[:], csq[:])
            cri = sq.tile([P, 1], F32, tag="cri")
            nc.vector.reciprocal(cri[:], csr[:])
            cnb = nb.tile([P, D], BF16, tag="cnb")
            nc.vector.tensor_scalar_mul(cnb[:], cf[:], cri[:])
            cxp = xs.tile([P, TD, P], BF16, tag="xp")
            for j in range(TD):
                nc.tensor.transpose(cxp[:, j, :], cnb[:, j * P:(j + 1) * P],
                                    eye[:])
            nc.scalar.copy(ctxT[:, b, :, :], cxp[:])

        # ---- interleaved: per 4-tile group, prep then 4 b-chunk matmuls ----
        for g in range(NG):
            for dt in range(GM):
                t = g * GM + dt
                mf = mp.tile([P, D], F32, tag="mf")
                nc.sync.dma_start(mf[:], mem_dram[t * P:(t + 1) * P, :])
                dump = sq.tile([P, D], BF16, tag="dump")
                nc.scalar.activation(dump[:], mf[:],
                                     mybir.ActivationFunctionType.Square,
                                     accum_out=ssq[:, t:t + 1])
                nc.scalar.sqrt(srt[:, t:t + 1], ssq[:, t:t + 1])
                nc.vector.reciprocal(rin[:, t:t + 1], srt[:, t:t + 1])
                mnb = nb.tile([P, D], BF16, tag="mnb")
                nc.vector.tensor_scalar_mul(mnb[:], mf[:], rin[:, t:t + 1])
                mxp = xs.tile([P, TD, P], BF16, tag="xp")
                for j in range(TD):
                    nc.tensor.transpose(mxp[:, j, :],
                                        mnb[:, j * P:(j + 1) * P], eye[:])
                if t % 2 == 0:
                    nc.vector.tensor_copy(memT[:, t, :, :], mxp[:])
                else:
                    nc.scalar.copy(memT[:, t, :, :], mxp[:])
            for b in range(TB):
                acc = ps.tile([P, GM * P], F32, tag="acc")
                for j in range(TD):
                    nc.tensor.matmul(
                        acc[:],
                        ctxT[:, b, j, :],
                        memT[:, g * GM:(g + 1) * GM, j, :],
                        start=(j == 0), stop=(j == TD - 1),
                    )
                if (b + g) % 2 == 0:
                    nc.scalar.copy(simb[:, b, g * GM * P:(g + 1) * GM * P],
                                   acc[:])
                else:
                    nc.vector.tensor_copy(
                        simb[:, b, g * GM * P:(g + 1) * GM * P], acc[:])
            if g == NG // 2 - 1:
                for b in range(TB):
                    t8v = sm.tile([P, 8], BF16, tag="t8v")
                    t8i = sm.tile([P, 8], U32, tag="t8i")
                    nc.vector.max(t8v[:], simb[:, b, 0:SEG])
                    nc.vector.max_index(t8i[:], t8v[:], simb[:, b, 0:SEG])
                    nc.gpsimd.dma_start(cv_dram[b, 0], t8v[:])
                    nc.gpsimd.dma_start(ci_dram[b, 0], t8i[:])

        # ---- top8 per (b-chunk, segment) ----
        for b in range(TB):
            for q in range(1, Q):
                t8v = sm.tile([P, 8], BF16, tag="t8v")
                t8i = sm.tile([P, 8], U32, tag="t8i")
                nc.vector.max(t8v[:], simb[:, b, q * SEG:(q + 1) * SEG])
                nc.vector.max_index(t8i[:], t8v[:],
                                    simb[:, b, q * SEG:(q + 1) * SEG])
                nc.gpsimd.dma_start(cv_dram[b, q], t8v[:])
                nc.gpsimd.dma_start(ci_dram[b, q], t8i[:])

    nc.compile()
    _NC_CACHE[key] = nc
    return nc


def run_device(context, memory, trace=False):
    nc = build_nc()
    eye = np.eye(P, dtype=ml_dtypes.bfloat16)
    in_maps = [
        {"ctx": np.ascontiguousarray(context),
         "mem": np.ascontiguousarray(memory[c * M:(c + 1) * M]),
         "eye": eye}
        for c in range(C)
    ]
    res = run_bass_kernel_spmd(nc, in_maps, list(range(C)), trace=trace)
    return res


def kernel(context: np.ndarray, memory: np.ndarray) -> np.ndarray:
    res = run_device(context, memory)
    # ---- host: gather candidates, exact fp64 re-rank, gather rows ----
    cand = np.full((B, C * Q * 8), -1, dtype=np.int64)
    for c in range(C):
        ci = res.results[c]["cand_i"].astype(np.int64)  # [TB, Q, P, 8]
        for bt in range(TB):
            for q in range(Q):
                cols = slice((c * Q + q) * 8, (c * Q + q) * 8 + 8)
                cand[bt * P:(bt + 1) * P, cols] = (
                    c * M + q * SEG + ci[bt, q])
    ctx64 = context.astype(np.float64)
    mem64 = memory.astype(np.float64)
    ctxn = ctx64 / np.sqrt(np.maximum((ctx64 * ctx64).sum(1, keepdims=True),
                                      1e-12))
    mnorm = np.sqrt(np.maximum((mem64 * mem64).sum(1), 1e-12))
    # cos[b, k] for candidate k of context b
    rows = mem64[cand]                                  # [B, K, D]
    cos = np.einsum("bd,bkd->bk", ctxn, rows) / mnorm[cand]
    # argmax with smallest-index tie-break
    best = np.empty(B, dtype=np.int64)
    for b in range(B):
        cb, vb = cand[b], cos[b]
        mx = vb.max()
        best[b] = cb[vb >= mx].min()
    return memory[best][None, :, :].astype(np.float32)



# revision 7
# speedup vs baseline: 4.4802x; 4.4802x over previous
"""Trainium2 Bass kernel for nn_LongTermMemory (retrieval_knn).

reference: best[b] = argmax_m cos(context[b], memory[m]); return
memory[best][None] -> [1, B, D].

Strategy (8 NeuronCores, memory sharded on M -> 8192 rows/core):
  Host prep: l2-normalize context and memory rows (cheap: 0.1% of FLOPs),
  cast to fp8-e4m3, and pack into the DoubleRow matmul layout
  (k = 256*t + 128*i + p).
  Device (per core): fp8 DoubleRow GEMM screening of all 512x8192 cosine
  sims, reduced on the fly into per-window maxima using all engines:
    - "A" tiles sim[b_low, m]:  DVE windowed max (32-row windows) straight
      from PSUM.
    - "B" tiles simT[m_low, b]: ACT drains PSUM->SBUF fp16, Pool (gpsimd)
      partition-max (128-row windows).
  Host: every window within MARGIN of a context's global screened max is
  re-ranked exactly in fp64 from the original fp32 inputs (fp8 screening
  only selects candidates; the final argmax is decided at fp64), then
  gather rows.
"""

import numpy as np
import ml_dtypes

import concourse.bacc as bacc
import concourse.tile as tile
from concourse import mybir
from concourse.bass_utils import run_bass_kernel_spmd

B, D, M_TOT = 512, 512, 65536
C = 8                      # cores
M = M_TOT // C             # 8192 rows per core
P = 128
NCH = 8                    # m-chunks of 1024 per core
CHM = M // NCH             # 1024
W = 32                     # A-window rows
NW_A = 512 // W            # 16 windows per A-group
F32 = mybir.dt.float32
F16 = mybir.dt.float16
FP8 = mybir.dt.float8e4
DR = mybir.MatmulPerfMode.DoubleRow
E4M3 = ml_dtypes.float8_e4m3

# chunk 0 is all-B (8 m-tiles); chunks 1..7 are half A-group / half B.
A_CHUNKS = list(range(1, NCH))          # chunks carrying an A-group
N_BROW = 4 + 2 * len(A_CHUNKS)          # pool-op output rows: 4 + 14 = 18

MARGIN = 0.02

_NC_CACHE = {}


def build_nc():
    key = "nc"
    if key in _NC_CACHE:
        return _NC_CACHE[key]
    from contextlib import ExitStack

    nc = bacc.Bacc("TRN2", target_bir_lowering=False, debug=False)
    ctx_dram = nc.dram_tensor("ctx8", [P, 2, 2, B], FP8, kind="ExternalInput")
    mem_dram = nc.dram_tensor("mem8", [NCH, P, 2, 2, CHM], FP8,
                              kind="ExternalInput")
    wa_dram = nc.dram_tensor("wmaxA", [P, len(A_CHUNKS), 2, 2, NW_A], F16,
                             kind="ExternalOutput")
    pb_dram = nc.dram_tensor("pmaxB", [1, N_BROW, 2, B], F16,
                             kind="ExternalOutput")

    with tile.TileContext(nc) as tc, ExitStack() as ex:
        big = ex.enter_context(tc.tile_pool(name="big", bufs=1))
        mp = ex.enter_context(tc.tile_pool(name="mp", bufs=2))
        dr = ex.enter_context(tc.tile_pool(name="dr", bufs=4))
        ps = ex.enter_context(tc.tile_pool(name="ps", bufs=4, space="PSUM"))

        ctx8 = big.tile([P, 2, 2, B], FP8)
        nc.sync.dma_start(ctx8[:], ctx_dram[:])
        wmaxA = big.tile([P, len(A_CHUNKS), 2, 2, NW_A], F16)
        pmaxB = big.tile([1, N_BROW, 2, B], F16)

        brow = 0
        for ch in range(NCH):
            memsb = mp.tile([P, 2, 2, CHM], FP8, tag="memsb")
            nc.sync.dma_start(memsb[:], mem_dram[ch])

            if ch in A_CHUNKS:
                # first 512 m of the chunk: A layout (sim[b_low, m])
                for pr in range(2):                      # bc pairs
                    acc = ps.tile([P, 2, 512], F32, tag="ps")
                    for hf in range(2):
                        bc = 2 * pr + hf
                        for t in range(2):
                            nc.tensor.matmul(
                                acc[:, hf],
                                ctx8[:, t, :, bc * P:(bc + 1) * P],
                                memsb[:, t, :, 0:512],
                                start=(t == 0), stop=(t == 1),
                                perf_mode=DR)
                    nc.vector.tensor_reduce(
                        wmaxA[:, ch - 1, pr],
                        acc[:].rearrange("p h (g w) -> p h g w", w=W),
                        axis=mybir.AxisListType.X, op=mybir.AluOpType.max)
                b_tiles = [4, 6]                          # m-tile pairs base
            else:
                b_tiles = [0, 2, 4, 6]

            # B layout (simT[m_low, b]): pairs of 128-row m-tiles
            for k0 in b_tiles:
                acc = ps.tile([P, 2, 512], F32, tag="ps")
                for hf in range(2):
                    k = k0 + hf
                    for t in range(2):
                        nc.tensor.matmul(
                            acc[:, hf],
                            memsb[:, t, :, k * P:(k + 1) * P],
                            ctx8[:, t],
                            start=(t == 0), stop=(t == 1),
                            perf_mode=DR)
                st = dr.tile([P, 2, 512], F16, tag="drain")
                nc.scalar.copy(st[:], acc[:])
                nc.gpsimd.tensor_reduce(
                    pmaxB[:, brow].rearrange("r h b -> r (h b)"),
                    st[:].rearrange("p h b -> p (h b)"),
                    axis=mybir.AxisListType.C, op=mybir.AluOpType.max)
                brow += 1

        nc.sync.dma_start(wa_dram[:], wmaxA[:])
        nc.sync.dma_start(pb_dram[:], pmaxB[:])

    nc.compile()
    _NC_CACHE[key] = nc
    return nc


def _pack_dr_T(xn8):
    """[N, 512 d] fp8 -> [p, t, i, N] DoubleRow layout (k = 256t+128i+p)."""
    n = xn8.shape[0]
    return np.ascontiguousarray(
        xn8.T.reshape(2, 2, P, n).transpose(2, 0, 1, 3))


def run_device(context, memory, trace=False):
    nc = build_nc()
    ctxn = context / np.sqrt(
        np.maximum((context.astype(np.float64) ** 2).sum(1, keepdims=True),
                   1e-12))
    memn = memory / np.sqrt(
        np.maximum((memory.astype(np.float64) ** 2).sum(1, keepdims=True),
                   1e-12))
    ctx8 = _pack_dr_T(ctxn.astype(E4M3))
    in_maps = []
    for c in range(C):
        shard = memn[c * M:(c + 1) * M].astype(E4M3)
        arr = _pack_dr_T(shard)                       # [p, t, i, 8192]
        mem8 = np.ascontiguousarray(
            arr.reshape(P, 2, 2, NCH, CHM).transpose(3, 0, 1, 2, 4))
        in_maps.append({"ctx8": ctx8, "mem8": mem8})
    return run_bass_kernel_spmd(nc, in_maps, list(range(C)), trace=trace)


def _window_tables():
    """Per-core window list: (m_start, m_len) plus value extractors."""
    a_windows = []          # (ch, pr, hf, w) -> m_start local
    for a, ch in enumerate(A_CHUNKS):
        for w in range(NW_A):
            a_windows.append(ch * CHM + w * W)
    b_windows = []          # (brow, hf) -> m_start local
    brow_tiles = []
    for ch in range(NCH):
        pairs = [4, 6] if ch in A_CHUNKS else [0, 2, 4, 6]
        for k0 in pairs:
            brow_tiles.append((ch, k0))
    for ch, k0 in brow_tiles:
        for hf in range(2):
            b_windows.append(ch * CHM + (k0 + hf) * P)
    return np.array(a_windows), np.array(b_windows)


def kernel(context: np.ndarray, memory: np.ndarray) -> np.ndarray:
    res = run_device(context, memory)
    a_starts, b_starts = _window_tables()      # local m offsets
    NA, NB = len(a_starts), len(b_starts)      # 112, 36 per core

    # assemble per-b window values: [B, C*(NA+NB)]
    vals = np.empty((B, C * (NA + NB)), dtype=np.float32)
    starts = np.empty(C * (NA + NB), dtype=np.int64)
    lens = np.empty(C * (NA + NB), dtype=np.int64)
    for c in range(C):
        wa = res.results[c]["wmaxA"].astype(np.float32)   # [P, 7, 2, 2, 16]
        pb = res.results[c]["pmaxB"].astype(np.float32)   # [1, 18, 2, B]
        o = c * (NA + NB)
        # A: value[b, a*16+w] with b = (2pr+hf)*128+p
        va = wa.transpose(2, 3, 0, 1, 4).reshape(B, NA)
        vals[:, o:o + NA] = va
        starts[o:o + NA] = c * M + a_starts
        lens[o:o + NA] = W
        vb = pb.reshape(NB, B).T                          # [B, 36]
        vals[:, o + NA:o + NA + NB] = vb
        starts[o + NA:o + NA + NB] = c * M + b_starts
        lens[o + NA:o + NA + NB] = P

    # exact fp64 re-rank of candidate windows
    ctx64 = context.astype(np.float64)
    mem64 = memory.astype(np.float64)
    ctxn = ctx64 / np.sqrt(np.maximum((ctx64 * ctx64).sum(1, keepdims=True),
                                      1e-12))
    mnorm = np.sqrt(np.maximum((mem64 * mem64).sum(1), 1e-12))
    gmax = vals.max(1)
    best = np.empty(B, dtype=np.int64)
    for b in range(B):
        sel = np.nonzero(vals[b] >= gmax[b] - MARGIN)[0]
        rows = np.concatenate(
            [starts[i] + np.arange(lens[i]) for i in sel])
        cos = (mem64[rows] @ ctxn[b]) / mnorm[rows]
        mx = cos.max()
        best[b] = rows[cos >= mx].min()
    return memory[best][None, :, :].astype(np.float32)


# revision 11
# speedup vs baseline: 4.6585x; 1.0398x over previous
"""Trainium2 Bass kernel for nn_LongTermMemory (retrieval_knn).

reference: best[b] = argmax_m cos(context[b], memory[m]); return
memory[best][None] -> [1, B, D].

Strategy (8 NeuronCores, memory sharded on M -> 8192 rows/core):
  Host prep: l2-normalize context and memory rows (cheap: 0.1% of FLOPs),
  cast to fp8-e4m3, and pack into the DoubleRow matmul layout
  (k = 256*t + 128*i + p).
  Device (per core): fp8 DoubleRow GEMM screening of all 512x8192 cosine
  sims, reduced on the fly into per-window statistics spread over all
  engines (three routes, mixed to balance engine load):
    - "A"    sim[b_low, m] tiles: DVE windowed max (32-row windows)
             straight from PSUM.
    - "Bold" simT[m_low, b] tiles: ACT exp-drain PSUM->SBUF bf16, Pool
             (gpsimd) partition-max (128-row windows of exp(sim)).
    - "B2"   simT[m_low, b] tiles: ACT exp-drain, then PE indicator-matmul
             window sums of exp (8-row windows), DVE drains the sums.
  exp is monotonic, so window exp-maxes / exp-sums upper-bound window
  maxes; the host converts via log/lambda + s0.
  Host: every window within MARGIN of a context's global screened max is
  re-ranked exactly in fp64 from the original fp32 inputs (fp8 screening
  only selects candidates; the final argmax is decided at fp64), then
  gather rows.
"""

import numpy as np
import ml_dtypes

import concourse.bacc as bacc
import concourse.tile as tile
from concourse import mybir
from concourse.bass_utils import run_bass_kernel_spmd

B, D, M_TOT = 512, 512, 65536
C = 8                      # cores
M = M_TOT // C             # 8192 rows per core
P = 128
NCH = 8                    # m-chunks of 1024 per core
CHM = M // NCH             # 1024
W = 32                     # A-window rows
NW_A = 512 // W            # 16 windows per A-group
WB2 = 8                    # B2-window rows
NW_B2 = P // WB2           # 16 windows per B2 m-tile
F32 = mybir.dt.float32
F16 = mybir.dt.float16
BF16 = mybir.dt.bfloat16
FP8 = mybir.dt.float8e4
DR = mybir.MatmulPerfMode.DoubleRow
E4M3 = ml_dtypes.float8_e4m3

LAM = 600.0                # exp sharpness
S0 = 0.2                   # exp offset: exp(LAM*(sim - S0))

# chunk compositions: two 512-m groups per 1024-m chunk.
CHUNKS = [("A", "Bold"), ("A", "Bold"), ("A", "Bold"), ("A", "Bold"),
          ("A", "Bold"), ("A", "Bold"), ("A", "B2"), ("B2", "B2")]
N_A = sum(r.count("A") for r in CHUNKS)            # 7
N_B2 = sum(r.count("B2") for r in CHUNKS)          # 3
N_BOLD = sum(r.count("Bold") for r in CHUNKS)      # 6
N_BROW = 2 * N_BOLD                                # pool-op rows (per pair)

MARGIN = 0.02

_NC_CACHE = {}


def build_nc():
    key = "nc"
    if key in _NC_CACHE:
        return _NC_CACHE[key]
    from contextlib import ExitStack

    nc = bacc.Bacc("TRN2", target_bir_lowering=False, debug=False)
    ctx_dram = nc.dram_tensor("ctx8", [P, 2, 2, B], FP8, kind="ExternalInput")
    mem_dram = nc.dram_tensor("mem8", [NCH, P, 2, 2, CHM], FP8,
                              kind="ExternalInput")
    ind_dram = nc.dram_tensor("ind", [P, NW_B2], BF16, kind="ExternalInput")
    wa_dram = nc.dram_tensor("wmaxA", [P, N_A, 2, 2, NW_A], F16,
                             kind="ExternalOutput")
    pb_dram = nc.dram_tensor("pmaxB", [1, N_BROW, 2, B], BF16,
                             kind="ExternalOutput")
    ew_dram = nc.dram_tensor("expw", [N_B2, P, B], F32, kind="ExternalOutput")

    with tile.TileContext(nc) as tc, ExitStack() as ex:
        big = ex.enter_context(tc.tile_pool(name="big", bufs=1))
        mp = ex.enter_context(tc.tile_pool(name="mp", bufs=3))
        dr = ex.enter_context(tc.tile_pool(name="dr", bufs=3))
        ps = ex.enter_context(tc.tile_pool(name="ps", bufs=3, space="PSUM"))
        p2 = ex.enter_context(tc.tile_pool(name="p2", bufs=2, space="PSUM"))

        ctx8 = big.tile([P, 2, 2, B], FP8)
        nc.sync.dma_start(ctx8[:], ctx_dram[:])
        ind = big.tile([P, NW_B2], BF16)
        nc.sync.dma_start(ind[:], ind_dram[:])
        wmaxA = big.tile([P, N_A, 2, 2, NW_A], F16)
        pmaxB = big.tile([1, N_BROW, 2, B], BF16)
        biast = big.tile([P, 1], F32)
        nc.vector.memset(biast[:], -LAM * S0)

        ia = ib = i2 = 0
        for ch in range(NCH):
            memsb = mp.tile([P, 2, 2, CHM], FP8, tag="memsb")
            nc.sync.dma_start(memsb[:], mem_dram[ch])

            for gi, route in enumerate(CHUNKS[ch]):
                ms = gi * 512               # m offset within chunk
                if route == "A":
                    for pr in range(2):
                        acc = ps.tile([P, 2, 512], F32, tag="ps")
                        for hf in range(2):
                            bc = 2 * pr + hf
                            for t in range(2):
                                nc.tensor.matmul(
                                    acc[:, hf],
                                    ctx8[:, t, :, bc * P:(bc + 1) * P],
                                    memsb[:, t, :, ms:ms + 512],
                                    start=(t == 0), stop=(t == 1),
                                    perf_mode=DR)
                        nc.vector.tensor_reduce(
                            wmaxA[:, ia, pr],
                            acc[:].rearrange("p h (g w) -> p h g w", w=W),
                            axis=mybir.AxisListType.X,
                            op=mybir.AluOpType.max)
                    ia += 1
                elif route == "Bold":
                    for pr in range(2):
                        acc = ps.tile([P, 2, 512], F32, tag="ps")
                        for hf in range(2):
                            k0 = ms + (2 * pr + hf) * P
                            for t in range(2):
                                nc.tensor.matmul(
                                    acc[:, hf],
                                    memsb[:, t, :, k0:k0 + P],
                                    ctx8[:, t],
                                    start=(t == 0), stop=(t == 1),
                                    perf_mode=DR)
                        st = dr.tile([P, 2, 512], BF16, tag="drain")
                        nc.scalar.activation(
                            st[:], acc[:], mybir.ActivationFunctionType.Exp,
                            scale=LAM, bias=biast[:])
                        nc.gpsimd.tensor_reduce(
                            pmaxB[:, ib].rearrange("r h b -> r (h b)"),
                            st[:].rearrange("p h b -> p (h b)"),
                            axis=mybir.AxisListType.C,
                            op=mybir.AluOpType.max)
                        ib += 1
                else:  # B2
                    ps2t = p2.tile([P, 512], F32, tag="p2")
                    for pr in range(2):
                        acc = ps.tile([P, 2, 512], F32, tag="ps")
                        for hf in range(2):
                            k0 = ms + (2 * pr + hf) * P
                            for t in range(2):
                                nc.tensor.matmul(
                                    acc[:, hf],
                                    memsb[:, t, :, k0:k0 + P],
                                    ctx8[:, t],
                                    start=(t == 0), stop=(t == 1),
                                    perf_mode=DR)
                        ex8 = dr.tile([P, 2, 512], BF16, tag="exps")
                        nc.scalar.activation(
                            ex8[:], acc[:], mybir.ActivationFunctionType.Exp,
                            scale=LAM, bias=biast[:])
                        for hf in range(2):
                            j = 2 * pr + hf
                            nc.tensor.matmul(
                                ps2t[32 * j:32 * j + NW_B2, :],
                                ind[:], ex8[:, hf],
                                start=True, stop=True,
                                tile_position=(0, 32 * j))
                    exw = dr.tile([P, 512], F32, tag="exw")
                    nc.vector.tensor_copy(exw[:], ps2t[:])
                    nc.sync.dma_start(ew_dram[i2], exw[:])
                    i2 += 1

        nc.sync.dma_start(wa_dram[:], wmaxA[:])
        nc.sync.dma_start(pb_dram[:], pmaxB[:])

    nc.compile()
    _NC_CACHE[key] = nc
    return nc


def _pack_dr_T(xn8):
    """[N, 512 d] fp8 -> [p, t, i, N] DoubleRow layout (k = 256t+128i+p)."""
    n = xn8.shape[0]
    return np.ascontiguousarray(
        xn8.T.reshape(2, 2, P, n).transpose(2, 0, 1, 3))


def run_device(context, memory, trace=False):
    nc = build_nc()
    ctxn = context / np.sqrt(
        np.maximum((context.astype(np.float64) ** 2).sum(1, keepdims=True),
                   1e-12))
    memn = memory / np.sqrt(
        np.maximum((memory.astype(np.float64) ** 2).sum(1, keepdims=True),
                   1e-12))
    ctx8 = _pack_dr_T(ctxn.astype(E4M3))
    ind = (np.arange(P)[:, None] // WB2 ==
           np.arange(NW_B2)[None, :]).astype(ml_dtypes.bfloat16)
    in_maps = []
    for c in range(C):
        shard = memn[c * M:(c + 1) * M].astype(E4M3)
        arr = _pack_dr_T(shard)                       # [p, t, i, 8192]
        mem8 = np.ascontiguousarray(
            arr.reshape(P, 2, 2, NCH, CHM).transpose(3, 0, 1, 2, 4))
        in_maps.append({"ctx8": ctx8, "mem8": mem8, "ind": ind})
    return run_bass_kernel_spmd(nc, in_maps, list(range(C)), trace=trace)


def _window_tables():
    """Per-core windows as (m_start, m_len) in emission order per route."""
    a_starts, b_starts, e_starts = [], [], []
    for ch in range(NCH):
        for gi, route in enumerate(CHUNKS[ch]):
            ms = ch * CHM + gi * 512
            if route == "A":
                for w in range(NW_A):
                    a_starts.append(ms + w * W)
            elif route == "Bold":
                for k in range(4):        # pair-major: (pr, hf)
                    b_starts.append(ms + k * P)
            else:
                for j in range(4):
                    for w in range(NW_B2):
                        e_starts.append(ms + j * P + w * WB2)
    return (np.array(a_starts), np.array(b_starts), np.array(e_starts))


def kernel(context: np.ndarray, memory: np.ndarray) -> np.ndarray:
    res = run_device(context, memory)
    a_st, b_st, e_st = _window_tables()
    NA, NB, NE = len(a_st), len(b_st), len(e_st)   # per core
    NWC = NA + NB + NE

    vals = np.empty((B, C * NWC), dtype=np.float32)
    starts = np.empty(C * NWC, dtype=np.int64)
    lens = np.empty(C * NWC, dtype=np.int64)
    with np.errstate(divide="ignore"):
        for c in range(C):
            r = res.results[c]
            o = c * NWC
            # A: wmaxA [P, N_A, 2, 2, NW_A]; b = (2pr+hf)*128 + p
            va = r["wmaxA"].astype(np.float32)
            vals[:, o:o + NA] = va.transpose(2, 3, 0, 1, 4).reshape(B, NA)
            starts[o:o + NA] = c * M + a_st
            lens[o:o + NA] = W
            # Bold: pmaxB [1, N_BROW, 2, B] of exp -> log/lam + s0
            vb = r["pmaxB"].astype(np.float32).reshape(NB, B).T
            vals[:, o + NA:o + NA + NB] = np.log(vb) / LAM + S0
            starts[o + NA:o + NA + NB] = c * M + b_st
            lens[o + NA:o + NA + NB] = P
            # B2: expw [N_B2, P, B]: partition 32j+w -> (m-tile j, window w)
            ve = r["expw"].astype(np.float32)
            ve = ve.reshape(N_B2, 4, 32, B)[:, :, :NW_B2]   # [g2, j, w, B]
            vals[:, o + NA + NB:o + NWC] = (
                np.log(ve.reshape(NE, B)).T / LAM + S0)
            starts[o + NA + NB:o + NWC] = c * M + e_st
            lens[o + NA + NB:o + NWC] = WB2

    # exact fp64 re-rank of candidate windows
    ctx64 = context.astype(np.float64)
    mem64 = memory.astype(np.float64)
    ctxn = ctx64 / np.sqrt(np.maximum((ctx64 * ctx64).sum(1, keepdims=True),
                                      1e-12))
    mnorm = np.sqrt(np.maximum((mem64 * mem64).sum(1), 1e-12))
    gmax = vals.max(1)
    best = np.empty(B, dtype=np.int64)
    for b in range(B):
        sel = np.nonzero(vals[b] >= gmax[b] - MARGIN)[0]
        rows = np.concatenate(
            [starts[i] + np.arange(lens[i]) for i in sel])
        cos = (mem64[rows] @ ctxn[b]) / mnorm[rows]
        mx = cos.max()
        best[b] = rows[cos >= mx].min()
    return memory[best][None, :, :].astype(np.float32)
